# revision 14
# baseline (speedup 1.0000x reference)
"""EdgeGraphConv on 8 Trainium2 NeuronCores (v4: raw-feature gather,
capped bins + pooled overflow).

Distribution: dst-range sharding. Core c owns destination nodes
[c*N/8, (c+1)*N/8). No collectives; output is a concatenation.

Linearity trick: mean_e(h0[src]+bn + ef*We+be) over dst =
  (S_raw @ W + EF*We + deg*(bn+be)) / max(deg,1)
where S_raw[dst] = sum nf[src_e]  (raw 128-dim features!),
      EF[dst]    = sum ef_e,  deg = in-degree (host metadata).
There is NO per-node pre-matmul: the kernel gathers raw node-feature
rows (256 B, fully used) and applies W_node per dst tile AFTER
aggregation (2 tiny matmuls per tile).

Gather: SWDGE dma_gather of 256B rows, round-robin over 4 SWDGE
queues; ~2.6 ns/descriptor, so descriptor COUNT is the wall. Binning
caps each (tile, src-quarter) bin at CAPB*128 slots; the overflow
edges (~2% of total) are pooled per (round, quarter) into BO blocks,
cutting slot padding from ~20% to ~7%. Overflow slots carry a
round-local dst label tt*128+u (fp16 keeps ints <= 2048 exact) and
match against R shifted iota rows in one DVE is_equal.

Single NEFF serves all 8 cores (per-core differences are pure data).
"""

import sys

for _p in ("/opt/trn_rl_repo", "/opt/pypackages"):
    if _p not in sys.path:
        sys.path.append(_p)

import ml_dtypes
import numpy as np

import concourse.bass as bass
import concourse.mybir as mybir
import concourse.tile as tile
from concourse import bacc, library_config
from concourse.bass_utils import run_bass_kernel_spmd

FP16 = np.float16
N_CORES = 8
P = 128
NGRP = 4           # src-value quarters (windows of CH rows)
CAPB = 4           # main blocks per (tile, quarter)
SUB = 3840         # idxs per dma_gather call (one (r,g) stream; 240 descs/engine)
NQ = 4             # SWDGE queues


def build_bass(K_in, F, TILES, R, BO, LMAX, PAD_N, CH):
    NR = TILES // R
    TPB = NGRP * CAPB                        # main blocks per tile
    GB = R * CAPB + BO                       # st blocks per quarter-round
    RBLK = NGRP * GB                         # st blocks per round
    SLOTS_R = RBLK * P                       # slots per round
    TOT_SLOTS = NR * SLOTS_R

    nc = bacc.Bacc("TRN2", target_bir_lowering=False, debug=False,
                   num_devices=N_CORES, num_swdge_queues=NQ)
    dt = mybir.dt

    nfr_d = nc.dram_tensor("nfr", [PAD_N, K_in], dt.float16, kind="ExternalInput")
    wn_d = nc.dram_tensor("wn", [K_in, F], dt.float16, kind="ExternalInput")
    idn_d = nc.dram_tensor("idn", [P, P], dt.float16, kind="ExternalInput")
    iot_d = nc.dram_tensor("iot", [R, P], dt.float16, kind="ExternalInput")
    we_d = nc.dram_tensor("we", [1, F], dt.float32, kind="ExternalInput")
    bb_d = nc.dram_tensor("bb", [1, F], dt.float32, kind="ExternalInput")
    idx_d = nc.dram_tensor("idx", [P, TOT_SLOTS // 16], dt.int16,
                           kind="ExternalInput")
    dstl_d = nc.dram_tensor("dstl", [P, TILES, TPB], dt.float16,
                            kind="ExternalInput")
    dsto_d = nc.dram_tensor("dsto", [P, NR, NGRP * BO], dt.float16,
                            kind="ExternalInput")
    efg_d = nc.dram_tensor("efg", [P, TILES, LMAX], dt.float16,
                           kind="ExternalInput")
    dg_d = nc.dram_tensor("dg", [P, TILES, 2], dt.float32, kind="ExternalInput")
    out_d = nc.dram_tensor("out", [TILES * P, F], dt.float32,
                           kind="ExternalOutput")

    mult = mybir.AluOpType.mult
    is_equal = mybir.AluOpType.is_equal

    def st_main(g, tt, b):
        return g * GB + tt * CAPB + b

    def st_ovf(g, b):
        return g * GB + R * CAPB + b

    with tile.TileContext(nc) as tc:
        with tc.tile_pool(name="meta", bufs=1) as meta, \
             tc.tile_pool(name="st", bufs=2) as pst, \
             tc.tile_pool(name="oh", bufs=4) as poh, \
             tc.tile_pool(name="oho", bufs=2) as poho, \
             tc.tile_pool(name="fin", bufs=2) as pfin, \
             tc.tile_pool(name="ps", bufs=3, space="PSUM") as pps, \
             tc.tile_pool(name="psT", bufs=2, space="PSUM") as ppsT, \
             tc.tile_pool(name="pso", bufs=2, space="PSUM") as ppso:
            nc.gpsimd.load_library(library_config.mlp)
            RCOL = SLOTS_R // 16          # idx columns per round
            idx_sb = meta.tile([P, TOT_SLOTS // 16], dt.int16)
            nc.sync.dma_start(out=idx_sb[:, 0:RCOL], in_=idx_d.ap()[:, 0:RCOL])
            dstl_sb = meta.tile([P, TILES, TPB, 1], dt.float16)
            nc.sync.dma_start(out=dstl_sb[:, :, :, 0], in_=dstl_d.ap())
            dsto_sb = meta.tile([P, NR, NGRP * BO, 1, 1], dt.float16)
            nc.sync.dma_start(out=dsto_sb[:, :, :, 0, 0], in_=dsto_d.ap())
            efg_sb = meta.tile([P, TILES, LMAX], dt.float16)
            nc.sync.dma_start(out=efg_sb[:], in_=efg_d.ap())
            dg_sb = meta.tile([P, TILES, 2], dt.float32)
            nc.sync.dma_start(out=dg_sb[:], in_=dg_d.ap())
            wn_sb = meta.tile([K_in, F], dt.float16)
            nc.sync.dma_start(out=wn_sb[:], in_=wn_d.ap())
            idn_sb = meta.tile([P, P], dt.float16)
            nc.sync.dma_start(out=idn_sb[:], in_=idn_d.ap())
            # iota rows: iota_t[p, 0, tt, i] = tt*128 + i
            iota_t = meta.tile([P, 1, R, P], dt.float16)
            for tt in range(R):
                nc.sync.dma_start(
                    out=iota_t[:, 0, tt, :],
                    in_=iot_d.ap()[tt:tt + 1, :].partition_broadcast(P))
            web = meta.tile([P, 1, F], dt.float32)
            nc.sync.dma_start(out=web[:],
                              in_=we_d.ap()[0:1, :].partition_broadcast(P))
            bbb = meta.tile([P, 1, F], dt.float32)
            nc.sync.dma_start(out=bbb[:],
                              in_=bb_d.ap()[0:1, :].partition_broadcast(P))
            nc.sync.dma_start(out=idx_sb[:, RCOL:],
                              in_=idx_d.ap()[:, RCOL:])
            acc = meta.tile([P, TILES, K_in], dt.float16)
            efs = meta.tile([P, TILES, 1], dt.float32)

            # EF[dst] = sum of this dst's edge_feat values
            nc.vector.tensor_reduce(out=efs[:], in_=efg_sb[:],
                                    axis=mybir.AxisListType.X,
                                    op=mybir.AluOpType.add)

            qst = {"qc": 0}

            def emit_gathers(r, st):
                col = r * RCOL
                for g in range(NGRP):
                    n_g = GB * P
                    g0 = g * GB
                    base = g * CH
                    for s0 in range(0, n_g, SUB):
                        ns = min(SUB, n_g - s0)
                        nc.gpsimd.dma_gather(
                            out_ap=st[:, g0 + s0 // P: g0 + (s0 + ns) // P, :],
                            in_ap=nfr_d.ap()[base:PAD_N, :],
                            idxs_ap=idx_sb[:, col + s0 // 16:
                                           col + (s0 + ns) // 16],
                            num_idxs=ns, num_idxs_reg=ns,
                            elem_size=K_in,
                            queue_num=qst["qc"] % NQ,
                            single_packet=(ns <= 1024))
                        qst["qc"] += 1
                    col += n_g // 16

            for r in range(NR):
                st = pst.tile([P, RBLK, K_in], dt.float16, tag="st")
                emit_gathers(r, st)
                # overflow one-hots for this round, one DVE op:
                # oho[p, g, b, tt, i] = (dsto[p, r, g, b] == tt*128+i)
                oho = poho.tile([P, NGRP * BO, R, P], dt.float16, tag="oho")
                nc.vector.tensor_tensor(
                    out=oho[:],
                    in0=dsto_sb[:, r, :, :, :].to_broadcast(
                        [P, NGRP * BO, R, P]),
                    in1=iota_t[:].to_broadcast([P, NGRP * BO, R, P]),
                    op=is_equal)
                osb = pfin.tile([P, R, F], dt.float32, tag="osb")
                for tt in range(R):
                    t = r * R + tt
                    oh = poh.tile([P, TPB, P], dt.float16, tag="oh")
                    nc.vector.tensor_tensor(
                        out=oh[:],
                        in0=dstl_sb[:, t, :, :].to_broadcast([P, TPB, P]),
                        in1=iota_t[:, :, 0, :].to_broadcast([P, TPB, P]),
                        op=is_equal)
                    ps = pps.tile([P, K_in], dt.float32, tag="ps")
                    for j in range(TPB):
                        g, b = divmod(j, CAPB)
                        nc.tensor.matmul(
                            ps[:],
                            lhsT=oh[:, j, :],
                            rhs=st[:, st_main(g, tt, b), :],
                            start=(j == 0), stop=False)
                    for g in range(NGRP):
                        for b in range(BO):
                            last = (g == NGRP - 1 and b == BO - 1)
                            nc.tensor.matmul(
                                ps[:],
                                lhsT=oho[:, g * BO + b, tt, :],
                                rhs=st[:, st_ovf(g, b), :],
                                start=False, stop=last)
                    nc.scalar.copy(out=acc[:, t, :], in_=ps[:])
                    # finalize tile t inline: S_raw^T then @ W_node
                    psT = ppsT.tile([P, P], dt.float32, tag="psT")
                    nc.tensor.matmul(psT[:], lhsT=acc[:, t, :], rhs=idn_sb[:],
                                     start=True, stop=True)
                    hT = pfin.tile([P, P], dt.float16, tag="hT")
                    nc.scalar.copy(out=hT[:], in_=psT[:])
                    pso = ppso.tile([P, F], dt.float32, tag="pso")
                    nc.tensor.matmul(pso[:], lhsT=hT[:], rhs=wn_sb[:],
                                     start=True, stop=True)
                    nc.scalar.copy(out=osb[:, tt, :], in_=pso[:])
                # per-round epilogue + output store:
                # out = (S@W + EF*We + deg*(bn+be)) * rdeg
                t0 = r * R
                sl = slice(t0, t0 + R)
                t1 = pfin.tile([P, R, F], dt.float32, tag="t1")
                nc.vector.tensor_tensor(
                    out=t1[:],
                    in0=efs[:, sl, :].to_broadcast([P, R, F]),
                    in1=web[:].to_broadcast([P, R, F]),
                    op=mult)
                nc.vector.tensor_add(out=osb[:], in0=osb[:], in1=t1[:])
                nc.vector.tensor_tensor(
                    out=t1[:],
                    in0=dg_sb[:, sl, 0:1].to_broadcast([P, R, F]),
                    in1=bbb[:].to_broadcast([P, R, F]),
                    op=mult)
                nc.vector.tensor_add(out=osb[:], in0=osb[:], in1=t1[:])
                nc.vector.tensor_tensor(
                    out=osb[:], in0=osb[:],
                    in1=dg_sb[:, sl, 1:2].to_broadcast([P, R, F]),
                    op=mult)
                nc.sync.dma_start(
                    out=out_d.ap().rearrange("(p t) f -> p t f",
                                             t=TILES)[:, sl, :],
                    in_=osb[:])
    nc.compile()
    return nc


def _schedule(src, dst, edge_feat, n_nodes):
    """Host-side index-space binning by (core, dst-tile, src-quarter)
    with per-bin cap CAPB*128 and pooled per-(round, quarter) overflow."""
    RN = n_nodes // N_CORES
    TILES = (RN + P - 1) // P
    R = 1
    for d in range(1, TILES + 1):
        if TILES % d == 0 and d <= 7:
            R = d
    NR = TILES // R
    PAD_N = -(-n_nodes // P) * P
    CH = -(-PAD_N // NGRP)
    assert CH <= 32768

    core = dst // RN
    L = dst - core * RN
    t = L // P
    u = (L % P).astype(np.float32)
    g = src // CH
    key = (core * TILES + t) * NGRP + g
    order = np.lexsort((src, key))
    ss, us = src[order], u[order]
    nbins = N_CORES * TILES * NGRP
    cnt = np.bincount(key, minlength=nbins)
    starts = np.zeros(nbins + 1, dtype=np.int64)
    np.cumsum(cnt, out=starts[1:])
    CAP = CAPB * P
    # pooled overflow size per (core, round, quarter)
    ovf = np.maximum(cnt.reshape(N_CORES, TILES, NGRP) - CAP, 0)
    po = ovf.reshape(N_CORES, NR, R, NGRP).sum(axis=2)
    BO = max(1, int(np.ceil(po.max() / P)))

    GB = R * CAPB + BO
    SLOTS_R = NGRP * GB * P
    per_core = []
    for c in range(N_CORES):
        idxv = np.zeros(NR * SLOTS_R, dtype=np.int16)
        dstl = np.full((TILES, NGRP * CAPB, P), -1.0, dtype=np.float32)
        dsto = np.full((NR, NGRP, BO * P), -1.0, dtype=np.float32)
        for r in range(NR):
            for gg in range(NGRP):
                # main slots: [tile][CAP], then overflow pool [BO*P]
                p0 = r * SLOTS_R + gg * GB * P
                op_ = p0 + R * CAP
                no = 0
                for tt in range(R):
                    bi = (c * TILES + r * R + tt) * NGRP + gg
                    a, b = starts[bi], starts[bi + 1]
                    n = b - a
                    nm = min(n, CAP)
                    idxv[p0:p0 + nm] = (ss[a:a + nm] - gg * CH).astype(np.int16)
                    blkv = np.full(CAP, -1.0, dtype=np.float32)
                    blkv[:nm] = us[a:a + nm]
                    dstl[r * R + tt, gg * CAPB:(gg + 1) * CAPB, :] = \
                        blkv.reshape(CAPB, P)
                    if n > CAP:
                        k = n - CAP
                        idxv[op_ + no:op_ + no + k] = \
                            (ss[a + CAP:b] - gg * CH).astype(np.int16)
                        dsto[r, gg, no:no + k] = tt * P + us[a + CAP:b]
                        no += k
                    p0 += CAP
        per_core.append((
            idxv,
            dstl.transpose(2, 0, 1).astype(FP16).copy(),
            dsto.reshape(NR, NGRP * BO, P).transpose(2, 0, 1)
                .astype(FP16).copy(),
        ))
    return per_core, TILES, R, BO, NR, PAD_N, CH


def _pack_idx(idxv):
    """flat slot-ordered int16 idxs -> wrapped [P, n/16] (16-partition
    wrap, replicated to the 8 16-partition groups)."""
    w = idxv.reshape(-1, 16).T           # [16, n/16]
    return np.tile(w, (8, 1)).astype(np.int16)


def _run(node_feat, edge_feat, W_node, b_node, W_edge, b_edge, src, dst,
         trace=False):
    n_nodes, K_in = node_feat.shape
    F = W_node.shape[1]
    src = np.asarray(src, dtype=np.int64)
    dst = np.asarray(dst, dtype=np.int64)
    E = src.shape[0]

    per_core, TILES, R, BO, NR, PAD_N, CH = \
        _schedule(src, dst, edge_feat, n_nodes)
    RN = n_nodes // N_CORES

    nfr = np.zeros((PAD_N, K_in), dtype=FP16)
    nfr[:n_nodes] = node_feat.astype(FP16)

    # per-dst ef grid + degree (host index metadata). Device row of
    # global node n is core*(TILES*P) + (n - core*RN).
    deg = np.bincount(dst, minlength=n_nodes).astype(np.int64)
    LMAX = max(1, int(deg.max()))
    do = np.argsort(dst, kind="stable")
    dstart = np.zeros(n_nodes + 1, dtype=np.int64)
    np.cumsum(deg, out=dstart[1:])
    rank = np.arange(E, dtype=np.int64) - dstart[dst[do]]
    nn = np.arange(n_nodes, dtype=np.int64)
    pos = (nn // RN) * (TILES * P) + nn % RN      # node -> device row
    grid = np.zeros((N_CORES * TILES * P, LMAX), dtype=np.float32)
    grid[pos[dst[do]], rank] = edge_feat[do, 0]
    degp = np.zeros(N_CORES * TILES * P, dtype=np.float32)
    degp[pos] = deg

    nc = build_bass(K_in, F, TILES, R, BO, LMAX, PAD_N, CH)

    iot = np.arange(P, dtype=np.float32).reshape(1, P) + \
        np.arange(R, dtype=np.float32).reshape(R, 1) * P
    base_in = {
        "nfr": nfr,
        "wn": W_node.astype(FP16),
        "idn": np.eye(P, dtype=np.float32).astype(FP16),
        "iot": iot.astype(FP16),
        "we": W_edge.astype(np.float32).reshape(1, F),
        "bb": (b_node + b_edge).astype(np.float32).reshape(1, F),
    }
    in_maps = []
    for c in range(N_CORES):
        idxv, dstp, dsto = per_core[c]
        m = dict(base_in)
        m["idx"] = _pack_idx(idxv)
        m["dstl"] = dstp
        m["dsto"] = dsto
        gr = grid[c * TILES * P:(c + 1) * TILES * P]
        m["efg"] = gr.reshape(TILES, P, LMAX).transpose(1, 0, 2) \
                     .astype(FP16).copy()
        d = degp[c * TILES * P:(c + 1) * TILES * P].reshape(TILES, P)
        dgm = np.stack([d.T, 1.0 / np.maximum(d.T, 1.0)], axis=-1)
        m["dg"] = np.ascontiguousarray(dgm.astype(np.float32))
        in_maps.append(m)

    res = run_bass_kernel_spmd(nc, in_maps, core_ids=list(range(N_CORES)),
                               trace=trace)
    loc = np.arange(RN, dtype=np.int64)
    rows = (loc % P) * TILES + loc // P
    out = np.empty((n_nodes, F), dtype=np.float32)
    for c in range(N_CORES):
        out[c * RN:(c + 1) * RN] = res.results[c]["out"][rows]
    return out, res


def kernel(node_feat, edge_feat, W_node, b_node, W_edge, b_edge, src, dst):
    out, _ = _run(node_feat, edge_feat, W_node, b_node, W_edge, b_edge,
                  src, dst)
    return out


# revision 16
# speedup vs baseline: 1.1215x; 1.1215x over previous
"""EdgeGraphConv on 8 Trainium2 NeuronCores (v4: raw-feature gather,
capped bins + pooled overflow).

Distribution: dst-range sharding. Core c owns destination nodes
[c*N/8, (c+1)*N/8). No collectives; output is a concatenation.

Linearity trick: mean_e(h0[src]+bn + ef*We+be) over dst =
  (S_raw @ W + EF*We + deg*(bn+be)) / max(deg,1)
where S_raw[dst] = sum nf[src_e]  (raw 128-dim features!),
      EF[dst]    = sum ef_e,  deg = in-degree (host metadata).
There is NO per-node pre-matmul: the kernel gathers raw node-feature
rows (256 B, fully used) and applies W_node per dst tile AFTER
aggregation (2 tiny matmuls per tile).

Gather: SWDGE dma_gather of 256B rows, round-robin over 4 SWDGE
queues; ~2.6 ns/descriptor, so descriptor COUNT is the wall. Binning
caps each (tile, src-quarter) bin at CAPB*128 slots; the overflow
edges (~2% of total) are pooled per (round, quarter) into BO blocks,
cutting slot padding from ~20% to ~7%. Overflow slots carry a
round-local dst label tt*128+u (fp16 keeps ints <= 2048 exact) and
match against R shifted iota rows in one DVE is_equal.

Single NEFF serves all 8 cores (per-core differences are pure data).
"""

import sys

for _p in ("/opt/trn_rl_repo", "/opt/pypackages"):
    if _p not in sys.path:
        sys.path.append(_p)

import ml_dtypes
import numpy as np

import concourse.bass as bass
import concourse.mybir as mybir
import concourse.tile as tile
from concourse import bacc, library_config
from concourse.bass_utils import run_bass_kernel_spmd

FP16 = np.float16
N_CORES = 8
P = 128
NGRP = 4           # src-value quarters (windows of CH rows)
CAPB = 4           # main blocks per (tile, quarter)
SUB = 2048         # idxs per dma_gather call (128 descs/engine)
NQ = 4             # SWDGE queues


def build_bass(K_in, F, TILES, R, BOT, LMAX, PAD_N, CH):
    NR = TILES // R
    TPB = NGRP * CAPB                        # main blocks per tile
    # per-(round, quarter) block counts and offsets
    GBT = [[R * CAPB + BOT[r][g] for g in range(NGRP)] for r in range(NR)]
    RBLKT = [sum(GBT[r]) for r in range(NR)]
    RBLK = max(RBLKT)
    GOFF = [[sum(GBT[r][:g]) for g in range(NGRP)] for r in range(NR)]
    ROFF = [sum(RBLKT[:r]) for r in range(NR)]        # slot-block offsets
    QT = [sum(BOT[r]) for r in range(NR)]             # ovf blocks per round
    QMAX = max(max(QT), 1)
    TOT_SLOTS = sum(RBLKT) * P

    nc = bacc.Bacc("TRN2", target_bir_lowering=False, debug=False,
                   num_devices=N_CORES, num_swdge_queues=NQ)
    dt = mybir.dt

    nfr_d = nc.dram_tensor("nfr", [PAD_N, K_in], dt.float16, kind="ExternalInput")
    wn_d = nc.dram_tensor("wn", [K_in, F], dt.float16, kind="ExternalInput")
    idn_d = nc.dram_tensor("idn", [P, P], dt.float16, kind="ExternalInput")
    iot_d = nc.dram_tensor("iot", [R, P], dt.float16, kind="ExternalInput")
    we_d = nc.dram_tensor("we", [1, F], dt.float32, kind="ExternalInput")
    bb_d = nc.dram_tensor("bb", [1, F], dt.float32, kind="ExternalInput")
    idx_d = nc.dram_tensor("idx", [P, TOT_SLOTS // 16], dt.int16,
                           kind="ExternalInput")
    dstl_d = nc.dram_tensor("dstl", [P, TILES, TPB], dt.float16,
                            kind="ExternalInput")
    dsto_d = nc.dram_tensor("dsto", [P, NR, QMAX], dt.float16,
                            kind="ExternalInput")
    efg_d = nc.dram_tensor("efg", [P, TILES, LMAX], dt.float16,
                           kind="ExternalInput")
    dg_d = nc.dram_tensor("dg", [P, TILES, 2], dt.float32, kind="ExternalInput")
    out_d = nc.dram_tensor("out", [TILES * P, F], dt.float32,
                           kind="ExternalOutput")

    mult = mybir.AluOpType.mult
    is_equal = mybir.AluOpType.is_equal

    def st_main(r, g, tt, b):
        return GOFF[r][g] + tt * CAPB + b

    def st_ovf(r, g, b):
        return GOFF[r][g] + R * CAPB + b

    with tile.TileContext(nc) as tc:
        with tc.tile_pool(name="meta", bufs=1) as meta, \
             tc.tile_pool(name="st", bufs=2) as pst, \
             tc.tile_pool(name="oh", bufs=4) as poh, \
             tc.tile_pool(name="oho", bufs=2) as poho, \
             tc.tile_pool(name="fin", bufs=2) as pfin, \
             tc.tile_pool(name="ps", bufs=3, space="PSUM") as pps, \
             tc.tile_pool(name="psT", bufs=2, space="PSUM") as ppsT, \
             tc.tile_pool(name="pso", bufs=2, space="PSUM") as ppso:
            nc.gpsimd.load_library(library_config.mlp)
            RCOL = RBLKT[0] * P // 16     # idx columns of round 0
            idx_sb = meta.tile([P, TOT_SLOTS // 16], dt.int16)
            nc.sync.dma_start(out=idx_sb[:, 0:RCOL], in_=idx_d.ap()[:, 0:RCOL])
            dstl_sb = meta.tile([P, TILES, TPB, 1], dt.float16)
            nc.sync.dma_start(out=dstl_sb[:, :, :, 0], in_=dstl_d.ap())
            dsto_sb = meta.tile([P, NR, QMAX, 1, 1], dt.float16)
            nc.sync.dma_start(out=dsto_sb[:, :, :, 0, 0], in_=dsto_d.ap())
            efg_sb = meta.tile([P, TILES, LMAX], dt.float16)
            nc.sync.dma_start(out=efg_sb[:], in_=efg_d.ap())
            dg_sb = meta.tile([P, TILES, 2], dt.float32)
            nc.sync.dma_start(out=dg_sb[:], in_=dg_d.ap())
            wn_sb = meta.tile([K_in, F], dt.float16)
            nc.sync.dma_start(out=wn_sb[:], in_=wn_d.ap())
            idn_sb = meta.tile([P, P], dt.float16)
            nc.sync.dma_start(out=idn_sb[:], in_=idn_d.ap())
            # iota rows: iota_t[p, 0, tt, i] = tt*128 + i
            iota_t = meta.tile([P, 1, R, P], dt.float16)
            for tt in range(R):
                nc.sync.dma_start(
                    out=iota_t[:, 0, tt, :],
                    in_=iot_d.ap()[tt:tt + 1, :].partition_broadcast(P))
            web = meta.tile([P, 1, F], dt.float32)
            nc.sync.dma_start(out=web[:],
                              in_=we_d.ap()[0:1, :].partition_broadcast(P))
            bbb = meta.tile([P, 1, F], dt.float32)
            nc.sync.dma_start(out=bbb[:],
                              in_=bb_d.ap()[0:1, :].partition_broadcast(P))
            nc.sync.dma_start(out=idx_sb[:, RCOL:],
                              in_=idx_d.ap()[:, RCOL:])
            acc = meta.tile([P, TILES, K_in], dt.float16)
            efs = meta.tile([P, TILES, 1], dt.float32)

            # EF[dst] = sum of this dst's edge_feat values
            nc.vector.tensor_reduce(out=efs[:], in_=efg_sb[:],
                                    axis=mybir.AxisListType.X,
                                    op=mybir.AluOpType.add)

            qst = {"qc": 0}

            def emit_gathers(r, st):
                col = ROFF[r] * P // 16
                for g in range(NGRP):
                    n_g = GBT[r][g] * P
                    g0 = GOFF[r][g]
                    base = g * CH
                    for s0 in range(0, n_g, SUB):
                        ns = min(SUB, n_g - s0)
                        nc.gpsimd.dma_gather(
                            out_ap=st[:, g0 + s0 // P: g0 + (s0 + ns) // P, :],
                            in_ap=nfr_d.ap()[base:PAD_N, :],
                            idxs_ap=idx_sb[:, col + s0 // 16:
                                           col + (s0 + ns) // 16],
                            num_idxs=ns, num_idxs_reg=ns,
                            elem_size=K_in,
                            queue_num=qst["qc"] % NQ,
                            single_packet=(ns <= 1024))
                        qst["qc"] += 1
                    col += n_g // 16

            for r in range(NR):
                st = pst.tile([P, RBLK, K_in], dt.float16, tag="st")
                emit_gathers(r, st)
                # overflow one-hots for this round, one DVE op:
                # oho[p, q, tt, i] = (dsto[p, r, q] == tt*128+i)
                Q = QT[r]
                oho = poho.tile([P, QMAX, R, P], dt.float16, tag="oho")
                nc.vector.tensor_tensor(
                    out=oho[:, 0:Q, :, :],
                    in0=dsto_sb[:, r, 0:Q, :, :].to_broadcast([P, Q, R, P]),
                    in1=iota_t[:].to_broadcast([P, Q, R, P]),
                    op=is_equal)
                osb = pfin.tile([P, R, F], dt.float32, tag="osb")
                for tt in range(R):
                    t = r * R + tt
                    oh = poh.tile([P, TPB, P], dt.float16, tag="oh")
                    nc.vector.tensor_tensor(
                        out=oh[:],
                        in0=dstl_sb[:, t, :, :].to_broadcast([P, TPB, P]),
                        in1=iota_t[:, :, 0, :].to_broadcast([P, TPB, P]),
                        op=is_equal)
                    ps = pps.tile([P, K_in], dt.float32, tag="ps")
                    for j in range(TPB):
                        g, b = divmod(j, CAPB)
                        nc.tensor.matmul(
                            ps[:],
                            lhsT=oh[:, j, :],
                            rhs=st[:, st_main(r, g, tt, b), :],
                            start=(j == 0), stop=False)
                    q = 0
                    for g in range(NGRP):
                        for b in range(BOT[r][g]):
                            last = (q == QT[r] - 1)
                            nc.tensor.matmul(
                                ps[:],
                                lhsT=oho[:, q, tt, :],
                                rhs=st[:, st_ovf(r, g, b), :],
                                start=False, stop=last)
                            q += 1
                    nc.scalar.copy(out=acc[:, t, :], in_=ps[:])
                    # finalize tile t inline: S_raw^T then @ W_node
                    psT = ppsT.tile([P, P], dt.float32, tag="psT")
                    nc.tensor.matmul(psT[:], lhsT=acc[:, t, :], rhs=idn_sb[:],
                                     start=True, stop=True)
                    hT = pfin.tile([P, P], dt.float16, tag="hT")
                    nc.scalar.copy(out=hT[:], in_=psT[:])
                    pso = ppso.tile([P, F], dt.float32, tag="pso")
                    nc.tensor.matmul(pso[:], lhsT=hT[:], rhs=wn_sb[:],
                                     start=True, stop=True)
                    nc.scalar.copy(out=osb[:, tt, :], in_=pso[:])
                # per-round epilogue + output store:
                # out = (S@W + EF*We + deg*(bn+be)) * rdeg
                t0 = r * R
                sl = slice(t0, t0 + R)
                t1 = pfin.tile([P, R, F], dt.float32, tag="t1")
                nc.vector.tensor_tensor(
                    out=t1[:],
                    in0=efs[:, sl, :].to_broadcast([P, R, F]),
                    in1=web[:].to_broadcast([P, R, F]),
                    op=mult)
                nc.vector.tensor_add(out=osb[:], in0=osb[:], in1=t1[:])
                nc.vector.tensor_tensor(
                    out=t1[:],
                    in0=dg_sb[:, sl, 0:1].to_broadcast([P, R, F]),
                    in1=bbb[:].to_broadcast([P, R, F]),
                    op=mult)
                nc.vector.tensor_add(out=osb[:], in0=osb[:], in1=t1[:])
                nc.vector.tensor_tensor(
                    out=osb[:], in0=osb[:],
                    in1=dg_sb[:, sl, 1:2].to_broadcast([P, R, F]),
                    op=mult)
                nc.sync.dma_start(
                    out=out_d.ap().rearrange("(p t) f -> p t f",
                                             t=TILES)[:, sl, :],
                    in_=osb[:])
    nc.compile()
    return nc


def _schedule(src, dst, edge_feat, n_nodes):
    """Host-side index-space binning by (core, dst-tile, src-quarter)
    with per-bin cap CAPB*128 and pooled per-(round, quarter) overflow."""
    RN = n_nodes // N_CORES
    TILES = (RN + P - 1) // P
    R = 1
    for d in range(1, TILES + 1):
        if TILES % d == 0 and d <= 7:
            R = d
    NR = TILES // R
    PAD_N = -(-n_nodes // P) * P
    CH = -(-PAD_N // NGRP)
    assert CH <= 32768

    core = dst // RN
    L = dst - core * RN
    t = L // P
    u = (L % P).astype(np.float32)
    g = src // CH
    key = (core * TILES + t) * NGRP + g
    order = np.lexsort((src, key))
    ss, us = src[order], u[order]
    nbins = N_CORES * TILES * NGRP
    cnt = np.bincount(key, minlength=nbins)
    starts = np.zeros(nbins + 1, dtype=np.int64)
    np.cumsum(cnt, out=starts[1:])
    CAP = CAPB * P
    # pooled overflow size per (core, round, quarter) -> per-(r,g) blocks
    ovf = np.maximum(cnt.reshape(N_CORES, TILES, NGRP) - CAP, 0)
    po = ovf.reshape(N_CORES, NR, R, NGRP).sum(axis=2)   # [C, NR, NGRP]
    BOT = [[int(np.ceil(po[:, r, g].max() / P)) for g in range(NGRP)]
           for r in range(NR)]
    GBT = [[R * CAPB + BOT[r][g] for g in range(NGRP)] for r in range(NR)]
    RBLKT = [sum(GBT[r]) for r in range(NR)]
    ROFF = [sum(RBLKT[:r]) for r in range(NR)]
    QT = [sum(BOT[r]) for r in range(NR)]
    QMAX = max(max(QT), 1)
    TOT = sum(RBLKT) * P
    per_core = []
    for c in range(N_CORES):
        idxv = np.zeros(TOT, dtype=np.int16)
        dstl = np.full((TILES, NGRP * CAPB, P), -1.0, dtype=np.float32)
        dsto = np.full((NR, QMAX * P), -1.0, dtype=np.float32)
        for r in range(NR):
            qb = 0
            for gg in range(NGRP):
                p0 = (ROFF[r] + sum(GBT[r][:gg])) * P
                op_ = p0 + R * CAP
                no = 0
                for tt in range(R):
                    bi = (c * TILES + r * R + tt) * NGRP + gg
                    a, b = starts[bi], starts[bi + 1]
                    n = b - a
                    nm = min(n, CAP)
                    idxv[p0:p0 + nm] = (ss[a:a + nm] - gg * CH).astype(np.int16)
                    blkv = np.full(CAP, -1.0, dtype=np.float32)
                    blkv[:nm] = us[a:a + nm]
                    dstl[r * R + tt, gg * CAPB:(gg + 1) * CAPB, :] = \
                        blkv.reshape(CAPB, P)
                    if n > CAP:
                        k = n - CAP
                        idxv[op_ + no:op_ + no + k] = \
                            (ss[a + CAP:b] - gg * CH).astype(np.int16)
                        dsto[r, qb * P + no:qb * P + no + k] = \
                            tt * P + us[a + CAP:b]
                        no += k
                    p0 += CAP
                qb += BOT[r][gg]
        per_core.append((
            idxv,
            dstl.transpose(2, 0, 1).astype(FP16).copy(),
            dsto.reshape(NR, QMAX, P).transpose(2, 0, 1)
                .astype(FP16).copy(),
        ))
    return per_core, TILES, R, BOT, NR, PAD_N, CH


def _pack_idx(idxv):
    """flat slot-ordered int16 idxs -> wrapped [P, n/16] (16-partition
    wrap, replicated to the 8 16-partition groups)."""
    w = idxv.reshape(-1, 16).T           # [16, n/16]
    return np.tile(w, (8, 1)).astype(np.int16)


def _run(node_feat, edge_feat, W_node, b_node, W_edge, b_edge, src, dst,
         trace=False):
    n_nodes, K_in = node_feat.shape
    F = W_node.shape[1]
    src = np.asarray(src, dtype=np.int64)
    dst = np.asarray(dst, dtype=np.int64)
    E = src.shape[0]

    per_core, TILES, R, BOT, NR, PAD_N, CH = \
        _schedule(src, dst, edge_feat, n_nodes)
    RN = n_nodes // N_CORES

    nfr = np.zeros((PAD_N, K_in), dtype=FP16)
    nfr[:n_nodes] = node_feat.astype(FP16)

    # per-dst ef grid + degree (host index metadata). Device row of
    # global node n is core*(TILES*P) + (n - core*RN).
    deg = np.bincount(dst, minlength=n_nodes).astype(np.int64)
    LMAX = max(1, int(deg.max()))
    do = np.argsort(dst, kind="stable")
    dstart = np.zeros(n_nodes + 1, dtype=np.int64)
    np.cumsum(deg, out=dstart[1:])
    rank = np.arange(E, dtype=np.int64) - dstart[dst[do]]
    nn = np.arange(n_nodes, dtype=np.int64)
    pos = (nn // RN) * (TILES * P) + nn % RN      # node -> device row
    grid = np.zeros((N_CORES * TILES * P, LMAX), dtype=np.float32)
    grid[pos[dst[do]], rank] = edge_feat[do, 0]
    degp = np.zeros(N_CORES * TILES * P, dtype=np.float32)
    degp[pos] = deg

    nc = build_bass(K_in, F, TILES, R, BOT, LMAX, PAD_N, CH)

    iot = np.arange(P, dtype=np.float32).reshape(1, P) + \
        np.arange(R, dtype=np.float32).reshape(R, 1) * P
    base_in = {
        "nfr": nfr,
        "wn": W_node.astype(FP16),
        "idn": np.eye(P, dtype=np.float32).astype(FP16),
        "iot": iot.astype(FP16),
        "we": W_edge.astype(np.float32).reshape(1, F),
        "bb": (b_node + b_edge).astype(np.float32).reshape(1, F),
    }
    in_maps = []
    for c in range(N_CORES):
        idxv, dstp, dsto = per_core[c]
        m = dict(base_in)
        m["idx"] = _pack_idx(idxv)
        m["dstl"] = dstp
        m["dsto"] = dsto
        gr = grid[c * TILES * P:(c + 1) * TILES * P]
        m["efg"] = gr.reshape(TILES, P, LMAX).transpose(1, 0, 2) \
                     .astype(FP16).copy()
        d = degp[c * TILES * P:(c + 1) * TILES * P].reshape(TILES, P)
        dgm = np.stack([d.T, 1.0 / np.maximum(d.T, 1.0)], axis=-1)
        m["dg"] = np.ascontiguousarray(dgm.astype(np.float32))
        in_maps.append(m)

    res = run_bass_kernel_spmd(nc, in_maps, core_ids=list(range(N_CORES)),
                               trace=trace)
    loc = np.arange(RN, dtype=np.int64)
    rows = (loc % P) * TILES + loc // P
    out = np.empty((n_nodes, F), dtype=np.float32)
    for c in range(N_CORES):
        out[c * RN:(c + 1) * RN] = res.results[c]["out"][rows]
    return out, res


def kernel(node_feat, edge_feat, W_node, b_node, W_edge, b_edge, src, dst):
    out, _ = _run(node_feat, edge_feat, W_node, b_node, W_edge, b_edge,
                  src, dst)
    return out


# revision 17
# speedup vs baseline: 1.1551x; 1.0300x over previous
"""EdgeGraphConv on 8 Trainium2 NeuronCores (v4: raw-feature gather,
capped bins + pooled overflow).

Distribution: dst-range sharding. Core c owns destination nodes
[c*N/8, (c+1)*N/8). No collectives; output is a concatenation.

Linearity trick: mean_e(h0[src]+bn + ef*We+be) over dst =
  (S_raw @ W + EF*We + deg*(bn+be)) / max(deg,1)
where S_raw[dst] = sum nf[src_e]  (raw 128-dim features!),
      EF[dst]    = sum ef_e,  deg = in-degree (host metadata).
There is NO per-node pre-matmul: the kernel gathers raw node-feature
rows (256 B, fully used) and applies W_node per dst tile AFTER
aggregation (2 tiny matmuls per tile).

Gather: SWDGE dma_gather of 256B rows, round-robin over 4 SWDGE
queues; ~2.6 ns/descriptor, so descriptor COUNT is the wall. Binning
caps each (tile, src-quarter) bin at CAPB*128 slots; the overflow
edges (~2% of total) are pooled per (round, quarter) into BO blocks,
cutting slot padding from ~20% to ~7%. Overflow slots carry a
round-local dst label tt*128+u (fp16 keeps ints <= 2048 exact) and
match against R shifted iota rows in one DVE is_equal.

Single NEFF serves all 8 cores (per-core differences are pure data).
"""

import sys

for _p in ("/opt/trn_rl_repo", "/opt/pypackages"):
    if _p not in sys.path:
        sys.path.append(_p)

import ml_dtypes
import numpy as np

import concourse.bass as bass
import concourse.mybir as mybir
import concourse.tile as tile
from concourse import bacc, library_config
from concourse.bass_utils import run_bass_kernel_spmd

FP16 = np.float16
N_CORES = 8
P = 128
NGRP = 4           # src-value quarters (windows of CH rows)
CAPB = 4           # main blocks per (tile, quarter)
SUB = 2048         # idxs per dma_gather call (128 descs/engine)
NQ = 4             # SWDGE queues


def build_bass(K_in, F, TILES, R, BOT, LMAX, PAD_N, CH):
    NR = TILES // R
    TPB = NGRP * CAPB                        # main blocks per tile
    # per-(round, quarter) block counts and offsets
    GBT = [[R * CAPB + BOT[r][g] for g in range(NGRP)] for r in range(NR)]
    RBLKT = [sum(GBT[r]) for r in range(NR)]
    RBLK = max(RBLKT)
    GOFF = [[sum(GBT[r][:g]) for g in range(NGRP)] for r in range(NR)]
    ROFF = [sum(RBLKT[:r]) for r in range(NR)]        # slot-block offsets
    QT = [sum(BOT[r]) for r in range(NR)]             # ovf blocks per round
    QMAX = max(max(QT), 1)
    TOT_SLOTS = sum(RBLKT) * P

    nc = bacc.Bacc("TRN2", target_bir_lowering=False, debug=False,
                   num_devices=N_CORES, num_swdge_queues=NQ)
    dt = mybir.dt

    nfr_d = nc.dram_tensor("nfr", [PAD_N, K_in], dt.float16, kind="ExternalInput")
    wn_d = nc.dram_tensor("wn", [K_in, F], dt.float16, kind="ExternalInput")
    idn_d = nc.dram_tensor("idn", [P, P], dt.float16, kind="ExternalInput")
    iot_d = nc.dram_tensor("iot", [R, P], dt.float16, kind="ExternalInput")
    we_d = nc.dram_tensor("we", [1, F], dt.float32, kind="ExternalInput")
    bb_d = nc.dram_tensor("bb", [1, F], dt.float32, kind="ExternalInput")
    idx_d = nc.dram_tensor("idx", [P, TOT_SLOTS // 16], dt.int16,
                           kind="ExternalInput")
    dstl_d = nc.dram_tensor("dstl", [P, TILES, TPB], dt.float16,
                            kind="ExternalInput")
    dsto_d = nc.dram_tensor("dsto", [P, NR, QMAX], dt.float16,
                            kind="ExternalInput")
    efg_d = nc.dram_tensor("efg", [P, TILES, LMAX], dt.float16,
                           kind="ExternalInput")
    dg_d = nc.dram_tensor("dg", [P, TILES, 2], dt.float32, kind="ExternalInput")
    out_d = nc.dram_tensor("out", [TILES * P, F], dt.float32,
                           kind="ExternalOutput")

    mult = mybir.AluOpType.mult
    is_equal = mybir.AluOpType.is_equal

    def st_main(r, g, tt, b):
        return GOFF[r][g] + tt * CAPB + b

    def st_ovf(r, g, b):
        return GOFF[r][g] + R * CAPB + b

    with tile.TileContext(nc) as tc:
        with tc.tile_pool(name="meta", bufs=1) as meta, \
             tc.tile_pool(name="st", bufs=2) as pst, \
             tc.tile_pool(name="oh", bufs=4) as poh, \
             tc.tile_pool(name="oho", bufs=2) as poho, \
             tc.tile_pool(name="fin", bufs=2) as pfin, \
             tc.tile_pool(name="ps", bufs=3, space="PSUM") as pps, \
             tc.tile_pool(name="psT", bufs=2, space="PSUM") as ppsT, \
             tc.tile_pool(name="pso", bufs=2, space="PSUM") as ppso:
            nc.gpsimd.load_library(library_config.mlp)
            RCOL = RBLKT[0] * P // 16     # idx columns of round 0
            idx_sb = meta.tile([P, TOT_SLOTS // 16], dt.int16)
            nc.sync.dma_start(out=idx_sb[:, 0:RCOL], in_=idx_d.ap()[:, 0:RCOL])
            dstl_sb = meta.tile([P, TILES, TPB, 1], dt.float16)
            nc.sync.dma_start(out=dstl_sb[:, :, :, 0], in_=dstl_d.ap())
            dsto_sb = meta.tile([P, NR, QMAX, 1, 1], dt.float16)
            nc.sync.dma_start(out=dsto_sb[:, :, :, 0, 0], in_=dsto_d.ap())
            efg_sb = meta.tile([P, TILES, LMAX], dt.float16)
            nc.sync.dma_start(out=efg_sb[:], in_=efg_d.ap())
            dg_sb = meta.tile([P, TILES, 2], dt.float32)
            nc.sync.dma_start(out=dg_sb[:], in_=dg_d.ap())
            wn_sb = meta.tile([K_in, F], dt.float16)
            nc.sync.dma_start(out=wn_sb[:], in_=wn_d.ap())
            idn_sb = meta.tile([P, P], dt.float16)
            nc.sync.dma_start(out=idn_sb[:], in_=idn_d.ap())
            # iota rows: iota_t[p, 0, tt, i] = tt*128 + i
            iota_t = meta.tile([P, 1, R, P], dt.float16)
            for tt in range(R):
                nc.sync.dma_start(
                    out=iota_t[:, 0, tt, :],
                    in_=iot_d.ap()[tt:tt + 1, :].partition_broadcast(P))
            web = meta.tile([P, 1, F], dt.float32)
            nc.sync.dma_start(out=web[:],
                              in_=we_d.ap()[0:1, :].partition_broadcast(P))
            bbb = meta.tile([P, 1, F], dt.float32)
            nc.sync.dma_start(out=bbb[:],
                              in_=bb_d.ap()[0:1, :].partition_broadcast(P))
            nc.sync.dma_start(out=idx_sb[:, RCOL:],
                              in_=idx_d.ap()[:, RCOL:])
            acc = meta.tile([P, TILES, K_in], dt.float16)
            efs = meta.tile([P, TILES, 1], dt.float32)

            # EF[dst] = sum of this dst's edge_feat values
            nc.vector.tensor_reduce(out=efs[:], in_=efg_sb[:],
                                    axis=mybir.AxisListType.X,
                                    op=mybir.AluOpType.add)

            qst = {"qc": 0}

            def emit_gathers(r, st):
                col = ROFF[r] * P // 16
                for g in range(NGRP):
                    n_g = GBT[r][g] * P
                    g0 = GOFF[r][g]
                    base = g * CH
                    # split into two calls of <=124 descs/engine so two
                    # same-queue calls fit the 255-desc ring (no awaits)
                    half = (GBT[r][g] + 1) // 2 * P
                    for s0 in (0, half):
                        ns = min(half, n_g - s0)
                        nc.gpsimd.dma_gather(
                            out_ap=st[:, g0 + s0 // P: g0 + (s0 + ns) // P, :],
                            in_ap=nfr_d.ap()[base:PAD_N, :],
                            idxs_ap=idx_sb[:, col + s0 // 16:
                                           col + (s0 + ns) // 16],
                            num_idxs=ns, num_idxs_reg=ns,
                            elem_size=K_in,
                            queue_num=qst["qc"] % NQ,
                            single_packet=(ns <= 1024))
                        qst["qc"] += 1
                    col += n_g // 16

            for r in range(NR):
                st = pst.tile([P, RBLK, K_in], dt.float16, tag="st")
                emit_gathers(r, st)
                # overflow one-hots for this round, one DVE op:
                # oho[p, q, tt, i] = (dsto[p, r, q] == tt*128+i)
                Q = QT[r]
                oho = poho.tile([P, QMAX, R, P], dt.float16, tag="oho")
                nc.vector.tensor_tensor(
                    out=oho[:, 0:Q, :, :],
                    in0=dsto_sb[:, r, 0:Q, :, :].to_broadcast([P, Q, R, P]),
                    in1=iota_t[:].to_broadcast([P, Q, R, P]),
                    op=is_equal)
                osb = pfin.tile([P, R, F], dt.float32, tag="osb")
                for tt in range(R):
                    t = r * R + tt
                    oh = poh.tile([P, TPB, P], dt.float16, tag="oh")
                    nc.vector.tensor_tensor(
                        out=oh[:],
                        in0=dstl_sb[:, t, :, :].to_broadcast([P, TPB, P]),
                        in1=iota_t[:, :, 0, :].to_broadcast([P, TPB, P]),
                        op=is_equal)
                    ps = pps.tile([P, K_in], dt.float32, tag="ps")
                    for j in range(TPB):
                        g, b = divmod(j, CAPB)
                        nc.tensor.matmul(
                            ps[:],
                            lhsT=oh[:, j, :],
                            rhs=st[:, st_main(r, g, tt, b), :],
                            start=(j == 0), stop=False)
                    q = 0
                    for g in range(NGRP):
                        for b in range(BOT[r][g]):
                            last = (q == QT[r] - 1)
                            nc.tensor.matmul(
                                ps[:],
                                lhsT=oho[:, q, tt, :],
                                rhs=st[:, st_ovf(r, g, b), :],
                                start=False, stop=last)
                            q += 1
                    nc.scalar.copy(out=acc[:, t, :], in_=ps[:])
                    # finalize tile t inline: S_raw^T then @ W_node
                    psT = ppsT.tile([P, P], dt.float32, tag="psT")
                    nc.tensor.matmul(psT[:], lhsT=acc[:, t, :], rhs=idn_sb[:],
                                     start=True, stop=True)
                    hT = pfin.tile([P, P], dt.float16, tag="hT")
                    nc.scalar.copy(out=hT[:], in_=psT[:])
                    pso = ppso.tile([P, F], dt.float32, tag="pso")
                    nc.tensor.matmul(pso[:], lhsT=hT[:], rhs=wn_sb[:],
                                     start=True, stop=True)
                    nc.scalar.copy(out=osb[:, tt, :], in_=pso[:])
                # per-round epilogue + output store:
                # out = (S@W + EF*We + deg*(bn+be)) * rdeg
                t0 = r * R
                sl = slice(t0, t0 + R)
                t1 = pfin.tile([P, R, F], dt.float32, tag="t1")
                nc.vector.tensor_tensor(
                    out=t1[:],
                    in0=efs[:, sl, :].to_broadcast([P, R, F]),
                    in1=web[:].to_broadcast([P, R, F]),
                    op=mult)
                nc.vector.tensor_add(out=osb[:], in0=osb[:], in1=t1[:])
                nc.vector.tensor_tensor(
                    out=t1[:],
                    in0=dg_sb[:, sl, 0:1].to_broadcast([P, R, F]),
                    in1=bbb[:].to_broadcast([P, R, F]),
                    op=mult)
                nc.vector.tensor_add(out=osb[:], in0=osb[:], in1=t1[:])
                nc.vector.tensor_tensor(
                    out=osb[:], in0=osb[:],
                    in1=dg_sb[:, sl, 1:2].to_broadcast([P, R, F]),
                    op=mult)
                nc.sync.dma_start(
                    out=out_d.ap().rearrange("(p t) f -> p t f",
                                             t=TILES)[:, sl, :],
                    in_=osb[:])
    nc.compile()
    return nc


def _schedule(src, dst, edge_feat, n_nodes):
    """Host-side index-space binning by (core, dst-tile, src-quarter)
    with per-bin cap CAPB*128 and pooled per-(round, quarter) overflow."""
    RN = n_nodes // N_CORES
    TILES = (RN + P - 1) // P
    R = 1
    for d in range(1, TILES + 1):
        if TILES % d == 0 and d <= 7:
            R = d
    NR = TILES // R
    PAD_N = -(-n_nodes // P) * P
    CH = -(-PAD_N // NGRP)
    assert CH <= 32768

    core = dst // RN
    L = dst - core * RN
    t = L // P
    u = (L % P).astype(np.float32)
    g = src // CH
    key = (core * TILES + t) * NGRP + g
    order = np.lexsort((src, key))
    ss, us = src[order], u[order]
    nbins = N_CORES * TILES * NGRP
    cnt = np.bincount(key, minlength=nbins)
    starts = np.zeros(nbins + 1, dtype=np.int64)
    np.cumsum(cnt, out=starts[1:])
    CAP = CAPB * P
    # pooled overflow size per (core, round, quarter) -> per-(r,g) blocks
    ovf = np.maximum(cnt.reshape(N_CORES, TILES, NGRP) - CAP, 0)
    po = ovf.reshape(N_CORES, NR, R, NGRP).sum(axis=2)   # [C, NR, NGRP]
    BOT = [[int(np.ceil(po[:, r, g].max() / P)) for g in range(NGRP)]
           for r in range(NR)]
    GBT = [[R * CAPB + BOT[r][g] for g in range(NGRP)] for r in range(NR)]
    RBLKT = [sum(GBT[r]) for r in range(NR)]
    ROFF = [sum(RBLKT[:r]) for r in range(NR)]
    QT = [sum(BOT[r]) for r in range(NR)]
    QMAX = max(max(QT), 1)
    TOT = sum(RBLKT) * P
    per_core = []
    for c in range(N_CORES):
        idxv = np.zeros(TOT, dtype=np.int16)
        dstl = np.full((TILES, NGRP * CAPB, P), -1.0, dtype=np.float32)
        dsto = np.full((NR, QMAX * P), -1.0, dtype=np.float32)
        for r in range(NR):
            qb = 0
            for gg in range(NGRP):
                p0 = (ROFF[r] + sum(GBT[r][:gg])) * P
                op_ = p0 + R * CAP
                no = 0
                for tt in range(R):
                    bi = (c * TILES + r * R + tt) * NGRP + gg
                    a, b = starts[bi], starts[bi + 1]
                    n = b - a
                    nm = min(n, CAP)
                    idxv[p0:p0 + nm] = (ss[a:a + nm] - gg * CH).astype(np.int16)
                    blkv = np.full(CAP, -1.0, dtype=np.float32)
                    blkv[:nm] = us[a:a + nm]
                    dstl[r * R + tt, gg * CAPB:(gg + 1) * CAPB, :] = \
                        blkv.reshape(CAPB, P)
                    if n > CAP:
                        k = n - CAP
                        idxv[op_ + no:op_ + no + k] = \
                            (ss[a + CAP:b] - gg * CH).astype(np.int16)
                        dsto[r, qb * P + no:qb * P + no + k] = \
                            tt * P + us[a + CAP:b]
                        no += k
                    p0 += CAP
                qb += BOT[r][gg]
        per_core.append((
            idxv,
            dstl.transpose(2, 0, 1).astype(FP16).copy(),
            dsto.reshape(NR, QMAX, P).transpose(2, 0, 1)
                .astype(FP16).copy(),
        ))
    return per_core, TILES, R, BOT, NR, PAD_N, CH


def _pack_idx(idxv):
    """flat slot-ordered int16 idxs -> wrapped [P, n/16] (16-partition
    wrap, replicated to the 8 16-partition groups)."""
    w = idxv.reshape(-1, 16).T           # [16, n/16]
    return np.tile(w, (8, 1)).astype(np.int16)


def _run(node_feat, edge_feat, W_node, b_node, W_edge, b_edge, src, dst,
         trace=False):
    n_nodes, K_in = node_feat.shape
    F = W_node.shape[1]
    src = np.asarray(src, dtype=np.int64)
    dst = np.asarray(dst, dtype=np.int64)
    E = src.shape[0]

    per_core, TILES, R, BOT, NR, PAD_N, CH = \
        _schedule(src, dst, edge_feat, n_nodes)
    RN = n_nodes // N_CORES

    nfr = np.zeros((PAD_N, K_in), dtype=FP16)
    nfr[:n_nodes] = node_feat.astype(FP16)

    # per-dst ef grid + degree (host index metadata). Device row of
    # global node n is core*(TILES*P) + (n - core*RN).
    deg = np.bincount(dst, minlength=n_nodes).astype(np.int64)
    LMAX = max(1, int(deg.max()))
    do = np.argsort(dst, kind="stable")
    dstart = np.zeros(n_nodes + 1, dtype=np.int64)
    np.cumsum(deg, out=dstart[1:])
    rank = np.arange(E, dtype=np.int64) - dstart[dst[do]]
    nn = np.arange(n_nodes, dtype=np.int64)
    pos = (nn // RN) * (TILES * P) + nn % RN      # node -> device row
    grid = np.zeros((N_CORES * TILES * P, LMAX), dtype=np.float32)
    grid[pos[dst[do]], rank] = edge_feat[do, 0]
    degp = np.zeros(N_CORES * TILES * P, dtype=np.float32)
    degp[pos] = deg

    nc = build_bass(K_in, F, TILES, R, BOT, LMAX, PAD_N, CH)

    iot = np.arange(P, dtype=np.float32).reshape(1, P) + \
        np.arange(R, dtype=np.float32).reshape(R, 1) * P
    base_in = {
        "nfr": nfr,
        "wn": W_node.astype(FP16),
        "idn": np.eye(P, dtype=np.float32).astype(FP16),
        "iot": iot.astype(FP16),
        "we": W_edge.astype(np.float32).reshape(1, F),
        "bb": (b_node + b_edge).astype(np.float32).reshape(1, F),
    }
    in_maps = []
    for c in range(N_CORES):
        idxv, dstp, dsto = per_core[c]
        m = dict(base_in)
        m["idx"] = _pack_idx(idxv)
        m["dstl"] = dstp
        m["dsto"] = dsto
        gr = grid[c * TILES * P:(c + 1) * TILES * P]
        m["efg"] = gr.reshape(TILES, P, LMAX).transpose(1, 0, 2) \
                     .astype(FP16).copy()
        d = degp[c * TILES * P:(c + 1) * TILES * P].reshape(TILES, P)
        dgm = np.stack([d.T, 1.0 / np.maximum(d.T, 1.0)], axis=-1)
        m["dg"] = np.ascontiguousarray(dgm.astype(np.float32))
        in_maps.append(m)

    res = run_bass_kernel_spmd(nc, in_maps, core_ids=list(range(N_CORES)),
                               trace=trace)
    loc = np.arange(RN, dtype=np.int64)
    rows = (loc % P) * TILES + loc // P
    out = np.empty((n_nodes, F), dtype=np.float32)
    for c in range(N_CORES):
        out[c * RN:(c + 1) * RN] = res.results[c]["out"][rows]
    return out, res


def kernel(node_feat, edge_feat, W_node, b_node, W_edge, b_edge, src, dst):
    out, _ = _run(node_feat, edge_feat, W_node, b_node, W_edge, b_edge,
                  src, dst)
    return out


# revision 18
# speedup vs baseline: 1.1641x; 1.0077x over previous
"""EdgeGraphConv on 8 Trainium2 NeuronCores (v4: raw-feature gather,
capped bins + pooled overflow).

Distribution: dst-range sharding. Core c owns destination nodes
[c*N/8, (c+1)*N/8). No collectives; output is a concatenation.

Linearity trick: mean_e(h0[src]+bn + ef*We+be) over dst =
  (S_raw @ W + EF*We + deg*(bn+be)) / max(deg,1)
where S_raw[dst] = sum nf[src_e]  (raw 128-dim features!),
      EF[dst]    = sum ef_e,  deg = in-degree (host metadata).
There is NO per-node pre-matmul: the kernel gathers raw node-feature
rows (256 B, fully used) and applies W_node per dst tile AFTER
aggregation (2 tiny matmuls per tile).

Gather: SWDGE dma_gather of 256B rows, round-robin over 4 SWDGE
queues, each (round, quarter) stream split into two calls of <=120
descriptors/engine so two same-queue calls fit the 255-desc ring
(no completion awaits); ~2.3 ns/descriptor. Descriptor COUNT is the
wall. Binning
caps each (tile, src-quarter) bin at CAPB*128 slots; the overflow
edges (~2% of total) are pooled per (round, quarter) into BO blocks,
cutting slot padding from ~20% to ~7%. Overflow slots carry a
round-local dst label tt*128+u (fp16 keeps ints <= 2048 exact) and
match against R shifted iota rows in one DVE is_equal.

Single NEFF serves all 8 cores (per-core differences are pure data).
"""

import sys

for _p in ("/opt/trn_rl_repo", "/opt/pypackages"):
    if _p not in sys.path:
        sys.path.append(_p)

import ml_dtypes
import numpy as np

import concourse.bass as bass
import concourse.mybir as mybir
import concourse.tile as tile
from concourse import bacc, library_config
from concourse.bass_utils import run_bass_kernel_spmd

FP16 = np.float16
N_CORES = 8
P = 128
NGRP = 4           # src-value quarters (windows of CH rows)
CAPB = 4           # main blocks per (tile, quarter)
SUB = 2048         # idxs per dma_gather call (128 descs/engine)
NQ = 4             # SWDGE queues


def build_bass(K_in, F, TILES, R, BOT, LMAX, PAD_N, CH):
    NR = TILES // R
    TPB = NGRP * CAPB                        # main blocks per tile
    # per-(round, quarter) block counts and offsets
    GBT = [[R * CAPB + BOT[r][g] for g in range(NGRP)] for r in range(NR)]
    RBLKT = [sum(GBT[r]) for r in range(NR)]
    RBLK = max(RBLKT)
    GOFF = [[sum(GBT[r][:g]) for g in range(NGRP)] for r in range(NR)]
    ROFF = [sum(RBLKT[:r]) for r in range(NR)]        # slot-block offsets
    QT = [sum(BOT[r]) for r in range(NR)]             # ovf blocks per round
    QMAX = max(max(QT), 1)
    TOT_SLOTS = sum(RBLKT) * P

    nc = bacc.Bacc("TRN2", target_bir_lowering=False, debug=False,
                   num_devices=N_CORES, num_swdge_queues=NQ)
    dt = mybir.dt

    nfr_d = nc.dram_tensor("nfr", [PAD_N, K_in], dt.float16, kind="ExternalInput")
    wn_d = nc.dram_tensor("wn", [K_in, F], dt.float16, kind="ExternalInput")
    idn_d = nc.dram_tensor("idn", [P, P], dt.float16, kind="ExternalInput")
    iot_d = nc.dram_tensor("iot", [R, P], dt.float16, kind="ExternalInput")
    we_d = nc.dram_tensor("we", [1, F], dt.float32, kind="ExternalInput")
    bb_d = nc.dram_tensor("bb", [1, F], dt.float32, kind="ExternalInput")
    idx_d = nc.dram_tensor("idx", [P, TOT_SLOTS // 16], dt.int16,
                           kind="ExternalInput")
    dstl_d = nc.dram_tensor("dstl", [P, TILES, TPB], dt.float16,
                            kind="ExternalInput")
    dsto_d = nc.dram_tensor("dsto", [P, NR, QMAX], dt.float16,
                            kind="ExternalInput")
    efg_d = nc.dram_tensor("efg", [P, TILES, LMAX], dt.float16,
                           kind="ExternalInput")
    dg_d = nc.dram_tensor("dg", [P, TILES, 2], dt.float32, kind="ExternalInput")
    out_d = nc.dram_tensor("out", [TILES * P, F], dt.float32,
                           kind="ExternalOutput")

    mult = mybir.AluOpType.mult
    is_equal = mybir.AluOpType.is_equal

    def st_main(r, g, tt, b):
        return GOFF[r][g] + tt * CAPB + b

    def st_ovf(r, g, b):
        return GOFF[r][g] + R * CAPB + b

    with tile.TileContext(nc) as tc:
        with tc.tile_pool(name="meta", bufs=1) as meta, \
             tc.tile_pool(name="st", bufs=2) as pst, \
             tc.tile_pool(name="oh", bufs=4) as poh, \
             tc.tile_pool(name="oho", bufs=2) as poho, \
             tc.tile_pool(name="fin", bufs=2) as pfin, \
             tc.tile_pool(name="ps", bufs=3, space="PSUM") as pps, \
             tc.tile_pool(name="psT", bufs=2, space="PSUM") as ppsT, \
             tc.tile_pool(name="pso", bufs=2, space="PSUM") as ppso:
            nc.gpsimd.load_library(library_config.mlp)
            RCOL = RBLKT[0] * P // 16     # idx columns of round 0
            idx_sb = meta.tile([P, TOT_SLOTS // 16], dt.int16)
            nc.sync.dma_start(out=idx_sb[:, 0:RCOL], in_=idx_d.ap()[:, 0:RCOL])
            dstl_sb = meta.tile([P, TILES, TPB, 1], dt.float16)
            nc.sync.dma_start(out=dstl_sb[:, :, :, 0], in_=dstl_d.ap())
            dsto_sb = meta.tile([P, NR, QMAX, 1, 1], dt.float16)
            nc.sync.dma_start(out=dsto_sb[:, :, :, 0, 0], in_=dsto_d.ap())
            efg_sb = meta.tile([P, TILES, LMAX], dt.float16)
            nc.sync.dma_start(out=efg_sb[:], in_=efg_d.ap())
            dg_sb = meta.tile([P, TILES, 2], dt.float32)
            nc.sync.dma_start(out=dg_sb[:], in_=dg_d.ap())
            wn_sb = meta.tile([K_in, F], dt.float16)
            nc.sync.dma_start(out=wn_sb[:], in_=wn_d.ap())
            idn_sb = meta.tile([P, P], dt.float16)
            nc.sync.dma_start(out=idn_sb[:], in_=idn_d.ap())
            # iota rows: iota_t[p, 0, tt, i] = tt*128 + i
            iota_t = meta.tile([P, 1, R, P], dt.float16)
            for tt in range(R):
                nc.sync.dma_start(
                    out=iota_t[:, 0, tt, :],
                    in_=iot_d.ap()[tt:tt + 1, :].partition_broadcast(P))
            web = meta.tile([P, 1, F], dt.float32)
            nc.sync.dma_start(out=web[:],
                              in_=we_d.ap()[0:1, :].partition_broadcast(P))
            bbb = meta.tile([P, 1, F], dt.float32)
            nc.sync.dma_start(out=bbb[:],
                              in_=bb_d.ap()[0:1, :].partition_broadcast(P))
            nc.sync.dma_start(out=idx_sb[:, RCOL:],
                              in_=idx_d.ap()[:, RCOL:])
            acc = meta.tile([P, TILES, K_in], dt.float16)
            efs = meta.tile([P, TILES, 1], dt.float32)

            # EF[dst] = sum of this dst's edge_feat values
            nc.vector.tensor_reduce(out=efs[:], in_=efg_sb[:],
                                    axis=mybir.AxisListType.X,
                                    op=mybir.AluOpType.add)

            qst = {"qc": 0}

            def emit_gathers(r, st):
                col = ROFF[r] * P // 16
                for g in range(NGRP):
                    n_g = GBT[r][g] * P
                    g0 = GOFF[r][g]
                    base = g * CH
                    # split into two calls of <=124 descs/engine so two
                    # same-queue calls fit the 255-desc ring (no awaits)
                    half = (GBT[r][g] + 1) // 2 * P
                    for s0 in (0, half):
                        ns = min(half, n_g - s0)
                        nc.gpsimd.dma_gather(
                            out_ap=st[:, g0 + s0 // P: g0 + (s0 + ns) // P, :],
                            in_ap=nfr_d.ap()[base:PAD_N, :],
                            idxs_ap=idx_sb[:, col + s0 // 16:
                                           col + (s0 + ns) // 16],
                            num_idxs=ns, num_idxs_reg=ns,
                            elem_size=K_in,
                            queue_num=qst["qc"] % NQ,
                            single_packet=(ns <= 1024))
                        qst["qc"] += 1
                    col += n_g // 16

            for r in range(NR):
                st = pst.tile([P, RBLK, K_in], dt.float16, tag="st")
                emit_gathers(r, st)
                # overflow one-hots for this round, one DVE op:
                # oho[p, q, tt, i] = (dsto[p, r, q] == tt*128+i)
                Q = QT[r]
                oho = poho.tile([P, QMAX, R, P], dt.float16, tag="oho")
                nc.vector.tensor_tensor(
                    out=oho[:, 0:Q, :, :],
                    in0=dsto_sb[:, r, 0:Q, :, :].to_broadcast([P, Q, R, P]),
                    in1=iota_t[:].to_broadcast([P, Q, R, P]),
                    op=is_equal)
                osb = pfin.tile([P, R, F], dt.float32, tag="osb")
                for tt in range(R):
                    t = r * R + tt
                    oh = poh.tile([P, TPB, P], dt.float16, tag="oh")
                    nc.vector.tensor_tensor(
                        out=oh[:],
                        in0=dstl_sb[:, t, :, :].to_broadcast([P, TPB, P]),
                        in1=iota_t[:, :, 0, :].to_broadcast([P, TPB, P]),
                        op=is_equal)
                    ps = pps.tile([P, K_in], dt.float32, tag="ps")
                    for j in range(TPB):
                        g, b = divmod(j, CAPB)
                        nc.tensor.matmul(
                            ps[:],
                            lhsT=oh[:, j, :],
                            rhs=st[:, st_main(r, g, tt, b), :],
                            start=(j == 0), stop=False)
                    q = 0
                    for g in range(NGRP):
                        for b in range(BOT[r][g]):
                            last = (q == QT[r] - 1)
                            nc.tensor.matmul(
                                ps[:],
                                lhsT=oho[:, q, tt, :],
                                rhs=st[:, st_ovf(r, g, b), :],
                                start=False, stop=last)
                            q += 1
                    nc.scalar.copy(out=acc[:, t, :], in_=ps[:])
                    # finalize tile t inline: S_raw^T then @ W_node
                    psT = ppsT.tile([P, P], dt.float32, tag="psT")
                    nc.tensor.matmul(psT[:], lhsT=acc[:, t, :], rhs=idn_sb[:],
                                     start=True, stop=True)
                    hT = pfin.tile([P, P], dt.float16, tag="hT")
                    nc.scalar.copy(out=hT[:], in_=psT[:])
                    pso = ppso.tile([P, F], dt.float32, tag="pso")
                    nc.tensor.matmul(pso[:], lhsT=hT[:], rhs=wn_sb[:],
                                     start=True, stop=True)
                    nc.scalar.copy(out=osb[:, tt, :], in_=pso[:])
                # per-round epilogue + output store:
                # out = (S@W + EF*We + deg*(bn+be)) * rdeg
                t0 = r * R
                sl = slice(t0, t0 + R)
                t1 = pfin.tile([P, R, F], dt.float32, tag="t1")
                nc.vector.tensor_tensor(
                    out=t1[:],
                    in0=efs[:, sl, :].to_broadcast([P, R, F]),
                    in1=web[:].to_broadcast([P, R, F]),
                    op=mult)
                nc.vector.tensor_add(out=osb[:], in0=osb[:], in1=t1[:])
                nc.vector.tensor_tensor(
                    out=t1[:],
                    in0=dg_sb[:, sl, 0:1].to_broadcast([P, R, F]),
                    in1=bbb[:].to_broadcast([P, R, F]),
                    op=mult)
                nc.vector.tensor_add(out=osb[:], in0=osb[:], in1=t1[:])
                nc.vector.tensor_tensor(
                    out=osb[:], in0=osb[:],
                    in1=dg_sb[:, sl, 1:2].to_broadcast([P, R, F]),
                    op=mult)
                nc.sync.dma_start(
                    out=out_d.ap().rearrange("(p t) f -> p t f",
                                             t=TILES)[:, sl, :],
                    in_=osb[:])
    nc.compile()
    return nc


def _schedule(src, dst, edge_feat, n_nodes):
    """Host-side index-space binning by (core, dst-tile, src-quarter)
    with per-bin cap CAPB*128 and pooled per-(round, quarter) overflow."""
    RN = n_nodes // N_CORES
    TILES = (RN + P - 1) // P
    R = 1
    for d in range(1, TILES + 1):
        if TILES % d == 0 and d <= 7:
            R = d
    NR = TILES // R
    PAD_N = -(-n_nodes // P) * P
    CH = -(-PAD_N // NGRP)
    assert CH <= 32768

    core = dst // RN
    L = dst - core * RN
    t = L // P
    u = (L % P).astype(np.float32)
    g = src // CH
    key = (core * TILES + t) * NGRP + g
    order = np.lexsort((src, key))
    ss, us = src[order], u[order]
    nbins = N_CORES * TILES * NGRP
    cnt = np.bincount(key, minlength=nbins)
    starts = np.zeros(nbins + 1, dtype=np.int64)
    np.cumsum(cnt, out=starts[1:])
    CAP = CAPB * P
    # pooled overflow size per (core, round, quarter) -> per-(r,g) blocks
    ovf = np.maximum(cnt.reshape(N_CORES, TILES, NGRP) - CAP, 0)
    po = ovf.reshape(N_CORES, NR, R, NGRP).sum(axis=2)   # [C, NR, NGRP]
    BOT = [[int(np.ceil(po[:, r, g].max() / P)) for g in range(NGRP)]
           for r in range(NR)]
    GBT = [[R * CAPB + BOT[r][g] for g in range(NGRP)] for r in range(NR)]
    RBLKT = [sum(GBT[r]) for r in range(NR)]
    ROFF = [sum(RBLKT[:r]) for r in range(NR)]
    QT = [sum(BOT[r]) for r in range(NR)]
    QMAX = max(max(QT), 1)
    TOT = sum(RBLKT) * P
    per_core = []
    for c in range(N_CORES):
        idxv = np.zeros(TOT, dtype=np.int16)
        dstl = np.full((TILES, NGRP * CAPB, P), -1.0, dtype=np.float32)
        dsto = np.full((NR, QMAX * P), -1.0, dtype=np.float32)
        for r in range(NR):
            qb = 0
            for gg in range(NGRP):
                p0 = (ROFF[r] + sum(GBT[r][:gg])) * P
                op_ = p0 + R * CAP
                no = 0
                for tt in range(R):
                    bi = (c * TILES + r * R + tt) * NGRP + gg
                    a, b = starts[bi], starts[bi + 1]
                    n = b - a
                    nm = min(n, CAP)
                    idxv[p0:p0 + nm] = (ss[a:a + nm] - gg * CH).astype(np.int16)
                    blkv = np.full(CAP, -1.0, dtype=np.float32)
                    blkv[:nm] = us[a:a + nm]
                    dstl[r * R + tt, gg * CAPB:(gg + 1) * CAPB, :] = \
                        blkv.reshape(CAPB, P)
                    if n > CAP:
                        k = n - CAP
                        idxv[op_ + no:op_ + no + k] = \
                            (ss[a + CAP:b] - gg * CH).astype(np.int16)
                        dsto[r, qb * P + no:qb * P + no + k] = \
                            tt * P + us[a + CAP:b]
                        no += k
                    p0 += CAP
                qb += BOT[r][gg]
        per_core.append((
            idxv,
            dstl.transpose(2, 0, 1).astype(FP16).copy(),
            dsto.reshape(NR, QMAX, P).transpose(2, 0, 1)
                .astype(FP16).copy(),
        ))
    return per_core, TILES, R, BOT, NR, PAD_N, CH


def _pack_idx(idxv):
    """flat slot-ordered int16 idxs -> wrapped [P, n/16] (16-partition
    wrap, replicated to the 8 16-partition groups)."""
    w = idxv.reshape(-1, 16).T           # [16, n/16]
    return np.tile(w, (8, 1)).astype(np.int16)


def _run(node_feat, edge_feat, W_node, b_node, W_edge, b_edge, src, dst,
         trace=False):
    n_nodes, K_in = node_feat.shape
    F = W_node.shape[1]
    src = np.asarray(src, dtype=np.int64)
    dst = np.asarray(dst, dtype=np.int64)
    E = src.shape[0]

    per_core, TILES, R, BOT, NR, PAD_N, CH = \
        _schedule(src, dst, edge_feat, n_nodes)
    RN = n_nodes // N_CORES

    nfr = np.zeros((PAD_N, K_in), dtype=FP16)
    nfr[:n_nodes] = node_feat.astype(FP16)

    # per-dst ef grid + degree (host index metadata). Device row of
    # global node n is core*(TILES*P) + (n - core*RN).
    deg = np.bincount(dst, minlength=n_nodes).astype(np.int64)
    LMAX = max(1, int(deg.max()))
    do = np.argsort(dst, kind="stable")
    dstart = np.zeros(n_nodes + 1, dtype=np.int64)
    np.cumsum(deg, out=dstart[1:])
    rank = np.arange(E, dtype=np.int64) - dstart[dst[do]]
    nn = np.arange(n_nodes, dtype=np.int64)
    pos = (nn // RN) * (TILES * P) + nn % RN      # node -> device row
    grid = np.zeros((N_CORES * TILES * P, LMAX), dtype=np.float32)
    grid[pos[dst[do]], rank] = edge_feat[do, 0]
    degp = np.zeros(N_CORES * TILES * P, dtype=np.float32)
    degp[pos] = deg

    nc = build_bass(K_in, F, TILES, R, BOT, LMAX, PAD_N, CH)

    iot = np.arange(P, dtype=np.float32).reshape(1, P) + \
        np.arange(R, dtype=np.float32).reshape(R, 1) * P
    base_in = {
        "nfr": nfr,
        "wn": W_node.astype(FP16),
        "idn": np.eye(P, dtype=np.float32).astype(FP16),
        "iot": iot.astype(FP16),
        "we": W_edge.astype(np.float32).reshape(1, F),
        "bb": (b_node + b_edge).astype(np.float32).reshape(1, F),
    }
    in_maps = []
    for c in range(N_CORES):
        idxv, dstp, dsto = per_core[c]
        m = dict(base_in)
        m["idx"] = _pack_idx(idxv)
        m["dstl"] = dstp
        m["dsto"] = dsto
        gr = grid[c * TILES * P:(c + 1) * TILES * P]
        m["efg"] = gr.reshape(TILES, P, LMAX).transpose(1, 0, 2) \
                     .astype(FP16).copy()
        d = degp[c * TILES * P:(c + 1) * TILES * P].reshape(TILES, P)
        dgm = np.stack([d.T, 1.0 / np.maximum(d.T, 1.0)], axis=-1)
        m["dg"] = np.ascontiguousarray(dgm.astype(np.float32))
        in_maps.append(m)

    res = run_bass_kernel_spmd(nc, in_maps, core_ids=list(range(N_CORES)),
                               trace=trace)
    loc = np.arange(RN, dtype=np.int64)
    rows = (loc % P) * TILES + loc // P
    out = np.empty((n_nodes, F), dtype=np.float32)
    for c in range(N_CORES):
        out[c * RN:(c + 1) * RN] = res.results[c]["out"][rows]
    return out, res


def kernel(node_feat, edge_feat, W_node, b_node, W_edge, b_edge, src, dst):
    out, _ = _run(node_feat, edge_feat, W_node, b_node, W_edge, b_edge,
                  src, dst)
    return out
